# revision 17
# baseline (speedup 1.0000x reference)
"""Trainium2 Bass kernel for nn_CharEmbeddingV03x01 (dense_mlp).

Phase A: computes the full 256x7 output table on-chip, lands it in a DRAM
table with rows padded to 256B. Phase B: per-index gather via GPSIMD
dma_gather (SWDGE-generated 256B descriptors, single queue), DVE compaction
7-of-64, contiguous store. Pure data parallel across 8 cores.
"""

import sys

for _p in ("/opt/trn_rl_repo",):
    if _p not in sys.path:
        sys.path.insert(0, _p)

import numpy as np

NUM_CLASSES = 256
DIM_EMB = 7
B, L = 256, 2048
N_CORES = 8
TOK = B * L
TOK_DEV = TOK // N_CORES        # 65536 tokens per NeuronCore
TOK_GRP = TOK_DEV // 8          # 8192 tokens per Q7 core (8 Q7 cores/NC)
CHUNK = 2048                    # tokens per Q7 core per ap_gather instruction
N_CHUNK = TOK_GRP // CHUNK      # 4
IDS_S = TOK_GRP // 16           # 512 idx slots per SBUF partition

_BUILD_CACHE = {}


def _build_nc():
    from contextlib import ExitStack

    from concourse import bacc, mybir, tile

    f32 = mybir.dt.float32
    i16 = mybir.dt.int16
    AF = mybir.ActivationFunctionType
    ALU = mybir.AluOpType

    nc = bacc.Bacc("TRN2", target_bir_lowering=False, debug=False,
                   num_devices=N_CORES)

    idsw = nc.dram_tensor("idsw", [128, IDS_S], i16, kind="ExternalInput").ap()
    embt = nc.dram_tensor("embt", [7, 256], f32, kind="ExternalInput").ap()
    cstd = nc.dram_tensor("consts", [10, 96], f32, kind="ExternalInput").ap()
    out = nc.dram_tensor("out", [TOK_DEV * 7], f32, kind="ExternalOutput").ap()
    scratch = nc.dram_tensor("scratch", [1792], f32).ap()

    with tile.TileContext(nc) as tc, ExitStack() as ctx:
        const = ctx.enter_context(tc.tile_pool(name="const", bufs=1))
        work = ctx.enter_context(tc.tile_pool(name="work", bufs=1))
        psum = ctx.enter_context(tc.tile_pool(name="psum", bufs=1, space="PSUM"))
        outp = ctx.enter_context(tc.tile_pool(name="outp", bufs=2))

        ids16 = work.tile([128, IDS_S], i16)
        nc.sync.dma_start(ids16[:], idsw[:])

        embt_sb = const.tile([7, 256], f32)
        nc.sync.dma_start(embt_sb[:], embt[:])
        cst = const.tile([10, 96], f32)
        nc.sync.dma_start(cst[:], cstd[:])
        l1t = cst[0:7, 0:14]
        b1a = cst[0:10, 14:15]
        b1b = cst[0:4, 15:16]
        b2t = cst[0:3, 16:17]
        l2t = cst[0:10, 17:20]
        selt = cst[0:4, 20:38]
        l3e = cst[0:7, 38:45]
        l3b = cst[0:4, 45:52]
        l3c = cst[0:3, 52:59]
        l3m = cst[0:6, 59:66]
        l3a_ = cst[0:6, 66:73]
        l3x = cst[0:6, 73:80]
        l3n = cst[0:6, 80:87]
        l3o = cst[0:1, 87:94]

        ones = const.tile([1, 256], f32)
        nc.vector.memset(ones[:], 1.0)

        # --- Phase A ------------------------------------------------------
        p1a = psum.tile([10, 256], f32)
        nc.tensor.matmul(p1a[:], lhsT=l1t[:, 0:10], rhs=embt_sb[:],
                         start=True, stop=True)
        s10 = work.tile([10, 256], f32)
        nc.scalar.activation(s10[:], p1a[:], AF.Tanh, bias=b1a)

        p1b = psum.tile([4, 256], f32)
        nc.tensor.matmul(p1b[:], lhsT=l1t[:, 10:14], rhs=embt_sb[:],
                         start=True, stop=True)
        t73_71 = work.tile([4, 256], f32)
        nc.scalar.activation(t73_71[:], p1b[:], AF.Tanh, bias=b1b)

        p2 = psum.tile([3, 256], f32)
        nc.tensor.matmul(p2[:], lhsT=l2t, rhs=s10[:], start=True, stop=True)
        t213151 = work.tile([3, 256], f32)
        nc.scalar.activation(t213151[:], p2[:], AF.Tanh, bias=b2t)

        pA = psum.tile([6, 256], f32)
        nc.tensor.matmul(pA[:], lhsT=selt[0:3, 6:12], rhs=t213151[:],
                         start=True, stop=True)
        pB = psum.tile([6, 256], f32)
        nc.tensor.matmul(pB[:], lhsT=selt[0:3, 12:18], rhs=t213151[:],
                         start=True, stop=False)
        nc.tensor.matmul(pB[:], lhsT=selt[:, 0:6], rhs=t73_71[:],
                         start=False, stop=True)
        a6 = work.tile([6, 256], f32)
        nc.scalar.copy(a6[:], pA[:])
        b6 = work.tile([6, 256], f32)
        nc.scalar.copy(b6[:], pB[:])
        tm = work.tile([6, 256], f32)
        nc.vector.tensor_tensor(tm[:], a6[:], b6[:], op=ALU.mult)
        ta = work.tile([6, 256], f32)
        nc.vector.tensor_tensor(ta[:], a6[:], b6[:], op=ALU.add)
        tx = work.tile([6, 256], f32)
        nc.vector.tensor_tensor(tx[:], a6[:], b6[:], op=ALU.max)
        tn = work.tile([6, 256], f32)
        nc.vector.tensor_tensor(tn[:], a6[:], b6[:], op=ALU.min)

        pieces = [
            (embt_sb, l3e), (t73_71, l3b), (t213151, l3c),
            (tm, l3m), (ta, l3a_), (tx, l3x), (tn, l3n), (ones, l3o),
        ]
        tabh = []
        for h in range(2):
            p3 = psum.tile([128, 7], f32, tag=f"p3_{h}")
            for gi, (src, w) in enumerate(pieces):
                nc.tensor.matmul(
                    p3[:], lhsT=src[:, h * 128:(h + 1) * 128], rhs=w,
                    start=(gi == 0), stop=(gi == len(pieces) - 1),
                )
            th = work.tile([128, 7], f32, tag=f"tab_half_{h}")
            nc.scalar.activation(th[:], p3[:], AF.Tanh)
            tabh.append(th)

        # flatten the table to DRAM, then replicate to the 16 partitions the
        # gathers are read out from (the other 112 partitions hold zeros).
        nc.sync.dma_start(scratch[0:896], tabh[0][:])
        nc.sync.dma_start(scratch[896:1792], tabh[1][:])

        tab = work.tile([128, 1792], f32)
        nc.vector.memset(tab[:], 0.0)
        for p in range(0, 128, 8):
            nc.sync.dma_start(tab[p:p + 1, :], scratch[:])

        # --- Phase B: gather + store -------------------------------------
        for c in range(N_CHUNK):
            og = outp.tile([128, CHUNK * 7], f32, tag="og")
            nc.gpsimd.ap_gather(
                out_ap=og[:],
                in_ap=tab[:],
                idxs_ap=ids16[:, c * (CHUNK // 16):(c + 1) * (CHUNK // 16)],
                channels=128,
                num_elems=256,
                d=7,
                num_idxs=CHUNK,
            )
            half = (CHUNK // 2) * 7  # 7168 f32 per half
            for k in range(8):
                for h in range(2):
                    srcp = og[16 * k + 8 * h:16 * k + 8 * h + 1,
                              h * half:(h + 1) * half]
                    dst0 = (k * TOK_GRP + c * CHUNK) * 7 + h * half
                    nc.sync.dma_start(out[dst0:dst0 + half], srcp)

    nc.finalize()
    return nc


def _host_prep(ids, emb, W72, b72, W73, b73, W75, b75,
               W21, b21, W31, b31, W51, b51, W71, b71, Wout, bout):
    f = np.float32
    ids = np.asarray(ids)
    emb = np.asarray(emb, dtype=f)
    W72, W73, W75, W71 = (np.asarray(x, dtype=f) for x in (W72, W73, W75, W71))
    W21, W31, W51 = (np.asarray(x, dtype=f) for x in (W21, W31, W51))

    embt = np.ascontiguousarray(emb.T)
    l1 = np.ascontiguousarray(
        np.concatenate([W72, W75, W73, W73, W71], axis=1), dtype=f)
    bias = np.concatenate(
        [b72, b75, b73, b73, b71, b21, b31, b51]).reshape(17, 1).astype(f)
    l2 = np.zeros((10, 3), dtype=f)
    l2[0:2, 0] = W21[:, 0]
    l2[2:7, 2] = W51[:, 0]
    l2[7:10, 1] = W31[:, 0]

    a_rows = [0, 0, 0, 1, 1, 2]
    b_rows = [1, 2, None, 2, None, None]
    sel = np.zeros((4, 18), dtype=f)
    for j in range(6):
        sel[a_rows[j], 6 + j] = 1.0
        if b_rows[j] is None:
            sel[3, j] = 1.0
        else:
            sel[b_rows[j], 12 + j] = 1.0

    Wout = np.asarray(Wout, dtype=f)
    l3 = np.empty((39, 7), dtype=f)
    l3[0:11] = Wout[0:11]
    l3[11] = Wout[13]
    l3[12] = Wout[12]
    l3[13] = Wout[11]
    l3[14:38] = Wout[14:38]
    l3[38] = np.asarray(bout, dtype=f)

    cpk = np.zeros((10, 96), dtype=f)
    cpk[0:7, 0:14] = l1
    cpk[0:10, 14:15] = bias[0:10]
    cpk[0:4, 15:16] = bias[10:14]
    cpk[0:3, 16:17] = bias[14:17]
    cpk[0:10, 17:20] = l2
    cpk[0:4, 20:38] = sel
    cpk[0:7, 38:45] = l3[0:7]
    cpk[0:4, 45:52] = l3[7:11]
    cpk[0:3, 52:59] = l3[11:14]
    cpk[0:6, 59:66] = l3[14:20]
    cpk[0:6, 66:73] = l3[20:26]
    cpk[0:6, 73:80] = l3[26:32]
    cpk[0:6, 80:87] = l3[32:38]
    cpk[0:1, 87:94] = l3[38:39]

    shared = dict(embt=embt, consts=cpk)

    flat = ids.reshape(-1)
    in_maps = []
    for i in range(N_CORES):
        shard = flat[i * TOK_DEV:(i + 1) * TOK_DEV]
        # wrap layout for ap_gather: token k*8192 + s*16 + q of this core
        # lands at partition 16k+q, slot s.
        w = np.ascontiguousarray(
            shard.reshape(8, IDS_S, 16).transpose(0, 2, 1).reshape(128, IDS_S)
        ).astype(np.int16)
        m = dict(shared)
        m["idsw"] = w
        in_maps.append(m)
    return in_maps


def kernel(ids, emb, W72, b72, W73, b73, W75, b75,
           W21, b21, W31, b31, W51, b51, W71, b71, Wout, bout,
           _trace=False, _trace_kwargs=None):
    from concourse.bass_utils import run_bass_kernel_spmd

    if "nc" not in _BUILD_CACHE:
        _BUILD_CACHE["nc"] = _build_nc()
    nc = _BUILD_CACHE["nc"]

    in_maps = _host_prep(ids, emb, W72, b72, W73, b73, W75, b75,
                         W21, b21, W31, b31, W51, b51, W71, b71, Wout, bout)

    kwargs = {}
    if _trace:
        kwargs["trace"] = True
        if _trace_kwargs:
            kwargs.update(_trace_kwargs)
    res = run_bass_kernel_spmd(nc, in_maps, core_ids=list(range(N_CORES)), **kwargs)

    full = np.concatenate(
        [np.asarray(res.results[i]["out"]).reshape(TOK_DEV, 7)
         for i in range(N_CORES)], axis=0)
    out = full.reshape(B, L, 7)
    if _trace:
        return out, res
    return out
